# revision 26
# baseline (speedup 1.0000x reference)
"""AdaAttNStar fused kernel for 8 TRN2 NeuronCores.

Algebraic collapse: the reference builds A = Q^T K ([B, N, N]) explicitly, but
  M  = V A^T         = (V K^T) Q        ([B,C,C] Gram @ [B,C,N])
  S2 = V^2 A^T - M^2 = (V^2 K^T) Q - M^2
so the whole computation reduces to channel Grams ([B,3,3]), per-channel
normalization statistics, and one elementwise epilogue over [B,C,N].

Distribution: stats/Grams are global reductions, and on-chip collectives have a
multi-microsecond latency floor, so every core redundantly computes the (cheap)
reduction phase from the full inputs and epilogues only its 1/8 slice of N.
Per-core output slices are concatenated on the host.

Layouts: big tensors live in SBUF as [128, (b, c, f=72)] with spatial index
s = p*72 + f; reductions over s happen free-dim-first (DVE per partition) then
across partitions with a ones-column matmul on the PE. Input DMAs are spread
across the sync/scalar/tensor HWDGE queues so their ~0.8us issue costs overlap.
"""

import numpy as np

import concourse.bass as bass
import concourse.bacc as bacc
import concourse.tile as tile
from concourse import mybir
from concourse.bass_utils import run_bass_kernel_spmd

B, C, N = 2, 3, 9216
P, F = 128, 72            # N = P * F
NCORES = 8
NS, FS = N // NCORES, F // NCORES     # 1152, 9
MTOT = B * N              # 18432 elements per channel for the norm stats
EPS = 1e-12
f32 = mybir.dt.float32
Alu = mybir.AluOpType
Act = mybir.ActivationFunctionType
X = mybir.AxisListType.X

FULL_INPUTS = ["F_c", "F_s", "F_c_previous", "F_s_previous"]

# acc / sc column maps.  acc (per-partition partials, [128, 84]):
#   0:36  Gram accums, order (w,b,i,j)   w=0: V*Fsp, w=1: V^2*Fsp
#  36:54  sums  per (t,b,c), t in (fc, fcp, fsp)
#  54:60  F_s sums  (b,i)   (= sv)
#  60:66  F_s sumsq (b,i)   (= sv2)
#  66:84  sumsq per (t,b,c)
SC_R = 0
SC_SUM = 36
SC_SV = 54
SC_SV2 = 60
SC_SS = 66
SC_UP = 84      # b-pooled sums, t in {fc,fcp,fsp}          [9]
SC_SSP = 93     # b-pooled sumsq                            [9]
SC_Q = 102      # UP^2                                      [9]
SC_VARM = 111   # SSP - Q/MTOT  (= var*(MTOT-1))            [9]
SC_STD = 120    # sqrt(VARM/(MTOT-1))                       [9]
SC_SE = 129     # STD + eps                                 [9]
SC_A = 138      # 1/(STD+eps): ac=138, aq=141, ak=144       [9]
SC_GRID = 147   # sv_w[b,i] * UP[fsp, j]                    [36]
SC_RC = 183     # R - GRID/MTOT                             [36]
SC_AKQ = 219    # ak*aq                                     [3]
SC_H = 222      # RC*akq_j     <- broadcast region start    [36]
SC_H0 = 258     # sum_j H*mq_j                              [12]
SC_AC2 = 270    # ac replicated per b                       [6]
SC_MCAC2 = 276  # mc*ac replicated per b                    [6]
SC_HM = 288     # scratch H*UP[fcp,j]/MTOT                  [36]
NBC = 60        # broadcast region size (SC_H .. SC_H+60)


def _body(tc, dr, out_sl):
    nc = tc.nc
    V = nc.vector
    with (
        tc.tile_pool(name="main", bufs=1) as pool,
        tc.tile_pool(name="scr", bufs=4) as scr,
        tc.tile_pool(name="ep", bufs=2) as ep,
        tc.tile_pool(name="ps", bufs=1, space="PSUM") as pp,
    ):
        # ---- input DMAs, spread across HWDGE queues ---------------------
        def load_full(name, key, eng):
            tl = pool.tile([P, B * C * F], f32, tag=key)
            eng.dma_start(
                tl[:].rearrange("p (b c f) -> p b c f", b=B, c=C),
                dr[name].ap())
            return tl

        t = {}
        t["fs"] = load_full("F_s", "fs", nc.sync)
        t["fsp"] = load_full("F_s_previous", "fsp", nc.scalar)
        t["fcp"] = load_full("F_c_previous", "fcp", nc.sync)
        t["fc"] = load_full("F_c", "fc", nc.sync)

        fcp_sl = pool.tile([P, B * C * FS], f32, tag="fcpsl")
        nc.sync.dma_start(
            fcp_sl[:].rearrange("p (b c f) -> p b c f", b=B, c=C),
            dr["F_cp_sl"].ap())
        fc_sl = pool.tile([P, B * C * FS], f32, tag="fcsl")
        nc.sync.dma_start(
            fc_sl[:].rearrange("p (b c f) -> p b c f", b=B, c=C),
            dr["F_c_sl"].ap())

        # F_cp slice replicated over i -> (b, i, j, f), built by gpsimd
        fcp3 = pool.tile([P, B * C * C * FS], f32, tag="fcp3")
        f3v = fcp3[:].rearrange("p (b i j f) -> p b i j f", b=B, i=C, j=C)
        fslv = fcp_sl[:].rearrange("p (b j f) -> p b j f", b=B, j=C)
        for b in range(B):
            for i in range(C):
                nc.gpsimd.tensor_copy(f3v[:, b, i], fslv[:, b])

        def v4(tl, f=F):   # [128, (b c f)] -> [128, b, c, f]
            return tl[:].rearrange("p (b c f) -> p b c f", b=B, c=C, f=f)

        # ---- squares (ACT) ---------------------------------------------
        v2 = pool.tile([P, B * C * F], f32, tag="v2")
        nc.scalar.activation(v2[:], t["fs"][:], Act.Square)
        sq = {"fs": v2}
        for key in ["fc", "fcp", "fsp"]:
            s = pool.tile([P, B * C * F], f32, tag=f"sq{key}")
            nc.scalar.activation(s[:], t[key][:], Act.Square)
            sq[key] = s

        ones_col = pool.tile([P, 1], f32, tag="onesc")
        nc.gpsimd.memset(ones_col[:], 1.0)
        ones_row = pool.tile([1, P], f32, tag="onesr")
        nc.gpsimd.memset(ones_row[:], 1.0)

        # ---- per-partition partial reductions (DVE) --------------------
        acc = pool.tile([P, 84], f32, tag="acc")
        fsv, v2v, fspv = v4(t["fs"]), v4(v2), v4(t["fsp"])
        for w, src in enumerate([fsv, v2v]):
            for b in range(B):
                for i in range(C):
                    for j in range(C):
                        q = ((w * B + b) * C + i) * C + j
                        wscr = scr.tile([P, F], f32, tag="wscr")
                        V.scalar_tensor_tensor(
                            out=wscr[:],
                            in0=src[:, b, i, :], scalar=1.0,
                            in1=fspv[:, b, j, :],
                            op0=Alu.mult, op1=Alu.mult,
                            accum_out=acc[:, q:q + 1])
        for ti, key in enumerate(["fc", "fcp", "fsp"]):
            V.reduce_sum(
                acc[:, 36 + ti * 6:42 + ti * 6].rearrange(
                    "p (b c) -> p b c", b=B), v4(t[key]), axis=X)
            V.reduce_sum(
                acc[:, 66 + ti * 6:72 + ti * 6].rearrange(
                    "p (b c) -> p b c", b=B), v4(sq[key]), axis=X)
        V.reduce_sum(acc[:, 54:60].rearrange("p (b c) -> p b c", b=B),
                     v4(t["fs"]), axis=X)
        V.reduce_sum(acc[:, 60:66].rearrange("p (b c) -> p b c", b=B),
                     v4(v2), axis=X)

        # ---- cross-partition reduction on the PE -----------------------
        p1 = pp.tile([1, 96], f32, tag="p1")
        nc.tensor.matmul(p1[:, 0:84], ones_col[:], acc[:, 0:84],
                         start=True, stop=True)

        # ---- scalar phase on [1, *] ------------------------------------
        sc = pool.tile([1, 384], f32, tag="sc")

        def scs(base, n):
            return sc[:, base:base + n]

        V.tensor_copy(scs(0, 84), p1[:, 0:84])
        sum_t = scs(SC_SUM, 18).rearrange("o (t b c) -> o t b c", t=3, b=B)
        ss_t = scs(SC_SS, 18).rearrange("o (t b c) -> o t b c", t=3, b=B)
        V.tensor_add(scs(SC_UP, 9), sum_t[:, :, 0, :], sum_t[:, :, 1, :])
        V.tensor_add(scs(SC_SSP, 9), ss_t[:, :, 0, :], ss_t[:, :, 1, :])
        V.tensor_mul(scs(SC_Q, 9), scs(SC_UP, 9), scs(SC_UP, 9))
        V.scalar_tensor_tensor(
            out=scs(SC_VARM, 9), in0=scs(SC_Q, 9), scalar=-1.0 / MTOT,
            in1=scs(SC_SSP, 9), op0=Alu.mult, op1=Alu.add)
        nc.scalar.activation(scs(SC_STD, 9), scs(SC_VARM, 9), Act.Sqrt,
                             scale=1.0 / (MTOT - 1))
        V.tensor_scalar_add(scs(SC_SE, 9), scs(SC_STD, 9), EPS)
        V.reciprocal(scs(SC_A, 9), scs(SC_SE, 9))

        # grid[w,b,i,j] = sv_w[b,i] * UP[fsp, j]; vw = sc[54:66] contiguous
        up_fsp = scs(SC_UP + 6, 3)
        V.tensor_mul(
            scs(SC_GRID, 36).rearrange("o (g j) -> o g j", j=C),
            scs(SC_SV, 12).unsqueeze(2).broadcast_to((1, 12, 3)),
            up_fsp.unsqueeze(1).broadcast_to((1, 12, 3)))
        V.scalar_tensor_tensor(
            out=scs(SC_RC, 36), in0=scs(SC_GRID, 36), scalar=-1.0 / MTOT,
            in1=scs(SC_R, 36), op0=Alu.mult, op1=Alu.add)
        V.tensor_mul(scs(SC_AKQ, 3), scs(SC_A + 6, 3), scs(SC_A + 3, 3))
        V.tensor_mul(
            scs(SC_H, 36).rearrange("o (g j) -> o g j", j=C),
            scs(SC_RC, 36).rearrange("o (g j) -> o g j", j=C),
            scs(SC_AKQ, 3).unsqueeze(1).broadcast_to((1, 12, 3)))
        V.scalar_tensor_tensor(
            out=scs(SC_HM, 36).rearrange("o (g j) -> o g j", j=C),
            in0=scs(SC_H, 36).rearrange("o (g j) -> o g j", j=C),
            scalar=1.0 / MTOT,
            in1=scs(SC_UP + 3, 3).unsqueeze(1).broadcast_to((1, 12, 3)),
            op0=Alu.mult, op1=Alu.mult)
        V.reduce_sum(scs(SC_H0, 12),
                     scs(SC_HM, 36).rearrange("o (g j) -> o g j", j=C), axis=X)
        V.tensor_copy(scs(SC_AC2, 6),
                      scs(SC_A, 3).unsqueeze(1).broadcast_to((1, 2, 3)))
        V.scalar_tensor_tensor(
            out=scs(SC_MCAC2, 6).rearrange("o (b c) -> o b c", b=B),
            in0=scs(SC_UP, 3).unsqueeze(1).broadcast_to((1, 2, 3)),
            scalar=1.0 / MTOT,
            in1=scs(SC_AC2, 6).rearrange("o (b c) -> o b c", b=B),
            op0=Alu.mult, op1=Alu.mult)

        # ---- broadcast the 60 epilogue scalars to all partitions -------
        pbc = pp.tile([P, 64], f32, tag="pbc")
        nc.tensor.matmul(pbc[:, 0:NBC], ones_row[:], scs(SC_H, NBC),
                         start=True, stop=True)
        bc = pool.tile([P, NBC], f32, tag="bc")
        V.tensor_copy(bc[:], pbc[:, 0:NBC])
        # bc cols: H (w,b,i,j) at w*18+b*9+i*3+j; h0 at 36+(w,b,i);
        #          ac2 at 48+(b,c); mcac2 at 54+(b,c)

        # ---- epilogue on this core's N-slice ---------------------------
        mt = {}
        for w in range(2):
            prod = ep.tile([P, B * C * C * FS], f32, tag="prod")
            V.tensor_mul(
                prod[:].rearrange("p (g f) -> p g f", f=FS),
                fcp3[:].rearrange("p (g f) -> p g f", f=FS),
                bc[:, w * 18:(w + 1) * 18].unsqueeze(2)
                .broadcast_to((P, 18, FS)))
            red = ep.tile([P, B * C * FS], f32, tag="red")
            V.reduce_sum(
                red[:].rearrange("p (g f) -> p g f", f=FS),
                prod[:].rearrange("p (g j f) -> p g f j", j=C, f=FS), axis=X)
            res = ep.tile([P, B * C * FS], f32, tag=f"mt{w}")
            V.scalar_tensor_tensor(
                out=res[:].rearrange("p (g f) -> p g f", f=FS),
                in0=bc[:, 36 + w * 6:42 + w * 6]
                .unsqueeze(2).broadcast_to((P, 6, FS)),
                scalar=-1.0,
                in1=red[:].rearrange("p (g f) -> p g f", f=FS),
                op0=Alu.mult, op1=Alu.add)
            mt[w] = res

        msq = ep.tile([P, B * C * FS], f32, tag="msq")
        nc.scalar.activation(msq[:], mt[0][:], Act.Square)
        s2 = ep.tile([P, B * C * FS], f32, tag="s2")
        V.tensor_sub(s2[:], mt[1][:], msq[:])
        s2c = ep.tile([P, B * C * FS], f32, tag="s2c")
        V.tensor_scalar_max(s2c[:], s2[:], 0.0)
        st = ep.tile([P, B * C * FS], f32, tag="st")
        nc.scalar.activation(st[:], s2c[:], Act.Sqrt)

        cnt = ep.tile([P, B * C * FS], f32, tag="cnt")
        V.tensor_mul(
            cnt[:].rearrange("p (g f) -> p g f", f=FS),
            fc_sl[:].rearrange("p (g f) -> p g f", f=FS),
            bc[:, 48:54].unsqueeze(2).broadcast_to((P, 6, FS)))
        V.scalar_tensor_tensor(
            out=cnt[:].rearrange("p (g f) -> p g f", f=FS),
            in0=bc[:, 54:60].unsqueeze(2).broadcast_to((P, 6, FS)),
            scalar=-1.0,
            in1=cnt[:].rearrange("p (g f) -> p g f", f=FS),
            op0=Alu.mult, op1=Alu.add)
        out_t = ep.tile([P, B * C * FS], f32, tag="outt")
        V.tensor_mul(out_t[:], st[:], cnt[:])
        V.tensor_add(out_t[:], out_t[:], mt[0][:])

        nc.sync.dma_start(
            out_sl.ap(),
            out_t[:].rearrange("p (b c f) -> p b c f", b=B, c=C))




# ---------------------------------------------------------------------------
# Raw-bacc implementation: manual semaphores, no TileContext. Saves the Tile
# entry barrier (~7us: DMAs start immediately) and the Tile exit sem-reset
# storm (~11us -> a per-engine barrier inc + one gpsimd range-clear).
# ---------------------------------------------------------------------------

def build_raw():
    nc = bacc.Bacc("TRN2", target_bir_lowering=False, debug=False,
                   num_devices=NCORES)
    # inputs/outputs are partition-major ([P, B, C, f]) so every DMA moves
    # contiguous 1728B/216B runs per partition; the host transposes.
    dr = {}
    for name in FULL_INPUTS:
        dr[name] = nc.dram_tensor(name, [P, B, C, F], f32,
                                  kind="ExternalInput")
    dr["F_cp_sl"] = nc.dram_tensor("F_cp_sl", [P, B, C, FS], f32,
                                   kind="ExternalInput")
    dr["F_c_sl"] = nc.dram_tensor("F_c_sl", [P, B, C, FS], f32,
                                  kind="ExternalInput")
    out_sl = nc.dram_tensor("out_sl", [P, B, C, FS], f32,
                            kind="ExternalOutput")

    sb = lambda name, shape: nc.alloc_sbuf_tensor(name, shape, f32).ap()
    t_fs = sb("t_fs", [P, B * C * F])
    t_fsp = sb("t_fsp", [P, B * C * F])
    t_fcp = sb("t_fcp", [P, B * C * F])
    t_fc = sb("t_fc", [P, B * C * F])
    v2 = sb("v2", [P, B * C * F])
    sq_fc = sb("sq_fc", [P, B * C * F])
    sq_fcp = sb("sq_fcp", [P, B * C * F])
    sq_fsp = sb("sq_fsp", [P, B * C * F])
    fcp_sl = sb("fcp_sl", [P, B * C * FS])
    fc_sl = sb("fc_sl", [P, B * C * FS])
    fcp3 = sb("fcp3", [P, B * C * C * FS])
    ones_col = sb("ones_col", [P, 1])
    ones_row = sb("ones_row", [1, P])
    acc = sb("acc", [P, 84])
    sc = sb("sc", [1, 384])
    bc = sb("bc", [P, NBC])
    wscr = [sb(f"wscr{k}", [P, F]) for k in range(36)]
    prod = [sb(f"prod{w}", [P, B * C * C * FS]) for w in range(2)]
    red = [sb(f"red{w}", [P, B * C * FS]) for w in range(2)]
    mt = [sb(f"mt{w}", [P, B * C * FS]) for w in range(2)]
    msq = sb("msq", [P, B * C * FS])
    s2 = sb("s2", [P, B * C * FS])
    s2c = sb("s2c", [P, B * C * FS])
    stt = sb("stt", [P, B * C * FS])
    cnt = sb("cnt", [P, B * C * FS])
    out_t = sb("out_t", [P, B * C * FS])
    p1 = nc.alloc_psum_tensor("p1", [1, 96], f32).ap()
    pbc = nc.alloc_psum_tensor("pbc", [P, 64], f32).ap()

    import contextlib
    ctx = contextlib.ExitStack()
    names = ["dFS", "dFSP", "dFCP", "dFC", "dSL1", "dSL2", "dOUT",
             "sACT", "sDVE", "sPE", "sGP", "sBAR"]
    S = {n: ctx.enter_context(nc.semaphore(n)) for n in names}
    (dFS, dFSP, dFCP, dFC, dSL1, dSL2, dOUT, sACT, sDVE, sPE, sGP,
     sBAR) = (S[n] for n in names)
    sems = list(S.values())

    def v4(ap_, f=F):
        return ap_.rearrange("p (b c f) -> p b c f", b=B, c=C, f=f)

    def gv(ap_, f=FS):
        return ap_.rearrange("p (g f) -> p g f", f=f)

    def scs(base, n):
        return sc[:, base:base + n]

    # every DVE op bumps sDVE; nd tracks the emission count so consumers
    # (and later same-engine dependents) can wait on exact values
    nd = [0]

    with nc.Block() as block:

        @block.sync
        def _(sync):
            sync.dma_start(v4(t_fs)[0:64], dr["F_s"].ap()[0:64]
                           ).then_inc(dFS, 16)
            sync.dma_start(v4(t_fsp)[0:64], dr["F_s_previous"].ap()[0:64]
                           ).then_inc(dFSP, 16)
            for ap_, name, sem in [
                    (v4(t_fcp), "F_c_previous", dFCP),
                    (v4(t_fc), "F_c", dFC),
                    (v4(fc_sl, FS), "F_c_sl", dSL2)]:
                sync.dma_start(ap_, dr[name].ap()).then_inc(sem, 16)
            sync.wait_ge(sDVE, ND_FINAL)
            sync.dma_start(out_sl.ap(), v4(out_t, FS)).then_inc(dOUT, 16)

        @block.scalar
        def _(scalar):
            scalar.dma_start(
                v4(t_fsp)[64:128], dr["F_s_previous"].ap()[64:128]
                ).then_inc(dFSP, 16)
            scalar.dma_start(
                v4(t_fs)[64:128], dr["F_s"].ap()[64:128]).then_inc(dFS, 16)
            scalar.dma_start(
                v4(fcp_sl, FS), dr["F_cp_sl"].ap()).then_inc(dSL1, 16)
            # const-ap memsets live at the head of the gpsimd stream; without
            # the init barrier the activation bias consts need this wait.
            scalar.wait_ge(sGP, 2)
            scalar.wait_ge(dFS, 32)
            scalar.activation(v2[:], t_fs[:], Act.Square).then_inc(sACT)  # 1
            scalar.wait_ge(dFSP, 32)
            scalar.activation(sq_fsp[:], t_fsp[:],
                              Act.Square).then_inc(sACT)  # 2
            scalar.wait_ge(dFCP, 16)
            scalar.activation(sq_fcp[:], t_fcp[:],
                              Act.Square).then_inc(sACT)  # 3
            scalar.wait_ge(dFC, 16)
            scalar.activation(sq_fc[:], t_fc[:], Act.Square).then_inc(sACT)  # 4
            scalar.wait_ge(sDVE, ND_VARM)
            scalar.activation(scs(SC_STD, 9), scs(SC_VARM, 9), Act.Sqrt,
                              scale=1.0 / (MTOT - 1)).then_inc(sACT)  # 5
            scalar.wait_ge(sDVE, ND_S2C)
            scalar.activation(stt[:], s2c[:], Act.Sqrt).then_inc(sACT)  # 6

        @block.gpsimd
        def _(gp):
            gp.memset(ones_col[:], 1.0).then_inc(sGP)  # 1
            gp.memset(ones_row[:], 1.0).then_inc(sGP)  # 2
            gp.wait_ge(dSL1, 16)
            fslv = fcp_sl.rearrange("p (b j f) -> p b j f", b=B, j=C)
            f3v = fcp3.rearrange("p (b i j f) -> p b i j f", b=B, i=C, j=C)
            for b in range(B):
                for i in range(C):
                    gp.tensor_copy(f3v[:, b, i], fslv[:, b]).then_inc(sGP)

        @block.vector
        def _(V):
            def dv(inst):
                nd[0] += 1
                inst.then_inc(sDVE, 1)
                return nd[0]

            def wv():
                V.wait_ge(sDVE, nd[0])

            V.wait_ge(dFS, 32)
            V.wait_ge(dFSP, 32)
            fsv, v2v, fspv = v4(t_fs), v4(v2), v4(t_fsp)
            for w, src in enumerate([fsv, v2v]):
                if w == 1:
                    V.wait_ge(sACT, 1)
                for b in range(B):
                    for i in range(C):
                        for j in range(C):
                            q = ((w * B + b) * C + i) * C + j
                            dv(V.scalar_tensor_tensor(
                                out=wscr[q][:],
                                in0=src[:, b, i, :], scalar=1.0,
                                in1=fspv[:, b, j, :],
                                op0=Alu.mult, op1=Alu.mult,
                                accum_out=acc[:, q:q + 1]))
            dv(V.reduce_sum(
                acc[:, 54:60].rearrange("p (b c) -> p b c", b=B),
                v4(t_fs), axis=X))
            dv(V.reduce_sum(
                acc[:, 60:66].rearrange("p (b c) -> p b c", b=B),
                v4(v2), axis=X))
            V.wait_ge(sACT, 4)
            V.wait_ge(dFC, 16)
            V.wait_ge(dFCP, 16)
            for ti, (raw, sqt) in enumerate(
                    [(t_fc, sq_fc), (t_fcp, sq_fcp), (t_fsp, sq_fsp)]):
                dv(V.reduce_sum(
                    acc[:, 36 + ti * 6:42 + ti * 6].rearrange(
                        "p (b c) -> p b c", b=B), v4(raw), axis=X))
                dv(V.reduce_sum(
                    acc[:, 66 + ti * 6:72 + ti * 6].rearrange(
                        "p (b c) -> p b c", b=B), v4(sqt), axis=X))
            assert nd[0] == ND_ACC

            # scalar phase: chained ops, explicit same-engine waits
            V.wait_ge(sPE, 1)
            dv(V.tensor_copy(scs(0, 84), p1[:, 0:84]))
            sum_t = scs(SC_SUM, 18).rearrange("o (t b c) -> o t b c", t=3, b=B)
            ss_t = scs(SC_SS, 18).rearrange("o (t b c) -> o t b c", t=3, b=B)
            wv()
            dv(V.tensor_add(scs(SC_UP, 9), sum_t[:, :, 0, :],
                            sum_t[:, :, 1, :]))
            wv()
            dv(V.tensor_add(scs(SC_SSP, 9), ss_t[:, :, 0, :],
                            ss_t[:, :, 1, :]))
            wv()
            dv(V.tensor_mul(scs(SC_Q, 9), scs(SC_UP, 9), scs(SC_UP, 9)))
            wv()
            dv(V.scalar_tensor_tensor(
                out=scs(SC_VARM, 9), in0=scs(SC_Q, 9), scalar=-1.0 / MTOT,
                in1=scs(SC_SSP, 9), op0=Alu.mult, op1=Alu.add))
            assert nd[0] == ND_VARM
            V.wait_ge(sACT, 5)
            dv(V.tensor_scalar_add(scs(SC_SE, 9), scs(SC_STD, 9), EPS))
            wv()
            dv(V.reciprocal(scs(SC_A, 9), scs(SC_SE, 9)))
            up_fsp = scs(SC_UP + 6, 3)
            wv()
            dv(V.tensor_mul(
                scs(SC_GRID, 36).rearrange("o (g j) -> o g j", j=C),
                scs(SC_SV, 12).unsqueeze(2).broadcast_to((1, 12, 3)),
                up_fsp.unsqueeze(1).broadcast_to((1, 12, 3))))
            wv()
            dv(V.scalar_tensor_tensor(
                out=scs(SC_RC, 36), in0=scs(SC_GRID, 36), scalar=-1.0 / MTOT,
                in1=scs(SC_R, 36), op0=Alu.mult, op1=Alu.add))
            wv()
            dv(V.tensor_mul(scs(SC_AKQ, 3), scs(SC_A + 6, 3),
                            scs(SC_A + 3, 3)))
            wv()
            dv(V.tensor_mul(
                scs(SC_H, 36).rearrange("o (g j) -> o g j", j=C),
                scs(SC_RC, 36).rearrange("o (g j) -> o g j", j=C),
                scs(SC_AKQ, 3).unsqueeze(1).broadcast_to((1, 12, 3))))
            wv()
            dv(V.scalar_tensor_tensor(
                out=scs(SC_HM, 36).rearrange("o (g j) -> o g j", j=C),
                in0=scs(SC_H, 36).rearrange("o (g j) -> o g j", j=C),
                scalar=1.0 / MTOT,
                in1=scs(SC_UP + 3, 3).unsqueeze(1).broadcast_to((1, 12, 3)),
                op0=Alu.mult, op1=Alu.mult))
            wv()
            dv(V.reduce_sum(
                scs(SC_H0, 12),
                scs(SC_HM, 36).rearrange("o (g j) -> o g j", j=C), axis=X))
            wv()
            dv(V.tensor_copy(
                scs(SC_AC2, 6),
                scs(SC_A, 3).unsqueeze(1).broadcast_to((1, 2, 3))))
            wv()
            dv(V.scalar_tensor_tensor(
                out=scs(SC_MCAC2, 6).rearrange("o (b c) -> o b c", b=B),
                in0=scs(SC_UP, 3).unsqueeze(1).broadcast_to((1, 2, 3)),
                scalar=1.0 / MTOT,
                in1=scs(SC_AC2, 6).rearrange("o (b c) -> o b c", b=B),
                op0=Alu.mult, op1=Alu.mult))
            assert nd[0] == ND_CHAIN

            # epilogue
            V.wait_ge(sPE, 2)
            dv(V.tensor_copy(bc[:], pbc[:, 0:NBC]))
            V.wait_ge(sGP, 8)
            for w in range(2):
                wv()
                dv(V.tensor_mul(
                    gv(prod[w]), gv(fcp3),
                    bc[:, w * 18:(w + 1) * 18].unsqueeze(2)
                    .broadcast_to((P, 18, FS))))
                wv()
                dv(V.reduce_sum(
                    gv(red[w]),
                    prod[w].rearrange("p (g j f) -> p g f j", j=C, f=FS),
                    axis=X))
                wv()
                k = dv(V.scalar_tensor_tensor(
                    out=gv(mt[w]),
                    in0=bc[:, 36 + w * 6:42 + w * 6]
                    .unsqueeze(2).broadcast_to((P, 6, FS)),
                    scalar=-1.0, in1=gv(red[w]),
                    op0=Alu.mult, op1=Alu.add))
                if w == 0:
                    assert k == ND_MT0
            wv()
            dv(V.tensor_mul(msq[:], mt[0][:], mt[0][:]))
            wv()
            dv(V.tensor_sub(s2[:], mt[1][:], msq[:]))
            wv()
            k = dv(V.tensor_scalar_max(s2c[:], s2[:], 0.0))
            assert k == ND_S2C
            V.wait_ge(dSL2, 16)
            wv()
            dv(V.tensor_mul(
                gv(cnt), gv(fc_sl),
                bc[:, 48:54].unsqueeze(2).broadcast_to((P, 6, FS))))
            wv()
            dv(V.scalar_tensor_tensor(
                out=gv(cnt),
                in0=bc[:, 54:60].unsqueeze(2).broadcast_to((P, 6, FS)),
                scalar=-1.0, in1=gv(cnt), op0=Alu.mult, op1=Alu.add))
            V.wait_ge(sACT, 6)
            wv()
            dv(V.tensor_mul(out_t[:], stt[:], cnt[:]))
            wv()
            k = dv(V.tensor_add(out_t[:], out_t[:], mt[0][:]))
            assert k == ND_FINAL

        @block.tensor
        def _(te):
            te.wait_ge(sGP, 1)
            te.wait_ge(sDVE, ND_ACC)
            te.matmul(p1[:, 0:84], ones_col[:], acc[:, 0:84],
                      start=True, stop=True).then_inc(sPE)
            te.wait_ge(sGP, 2)
            te.wait_ge(sDVE, ND_CHAIN)
            te.matmul(pbc[:, 0:NBC], ones_row[:], scs(SC_H, NBC),
                      start=True, stop=True).then_inc(sPE)

    # end-of-kernel: the Block exit already drained+barriered all engines;
    # gpsimd waits for the output DMA (overlapping the barrier with the
    # transfer), then resets kernel semaphores so the NEFF can re-execute.
    nc.gpsimd.wait_ge(dOUT, 16)
    lo = min(s.num for s in sems)
    hi = max(s.num for s in sems)
    nc.gpsimd.sem_clear(range(lo, hi + 1))

    ctx.pop_all()   # keep semaphores allocated
    nc.compile()
    return nc


# sDVE milestone values (every DVE op increments by 1; emission order above)
ND_ACC = 44            # 36 gram STTs + 8 stat reduces
ND_VARM = ND_ACC + 5   # copy, UP, SSP, Q, VARM
ND_CHAIN = ND_VARM + 10
ND_MT0 = ND_CHAIN + 4   # bc copy + prod0/red0/res0
ND_S2C = ND_CHAIN + 10  # + w=1 triple + msq + s2 + s2c
ND_FINAL = ND_S2C + 4   # cnt, cnt2, outmul, outadd


def build_tile():
    nc = bacc.Bacc("TRN2", target_bir_lowering=False, debug=False,
                   num_devices=NCORES)
    dr = {}
    for name in FULL_INPUTS:
        dr[name] = nc.dram_tensor(name, [P, B, C, F], f32,
                                  kind="ExternalInput")
    dr["F_cp_sl"] = nc.dram_tensor("F_cp_sl", [P, B, C, FS], f32,
                                   kind="ExternalInput")
    dr["F_c_sl"] = nc.dram_tensor("F_c_sl", [P, B, C, FS], f32,
                                  kind="ExternalInput")
    out_sl = nc.dram_tensor("out_sl", [P, B, C, FS], f32,
                            kind="ExternalOutput")
    with tile.TileContext(nc) as tc:
        _body(tc, dr, out_sl)
    nc.compile()
    return nc


import os as _os
def build():
    if _os.environ.get("KERNEL_IMPL", "raw") == "raw":
        return build_raw()
    return build_tile()


_NC = None


def _get_nc():
    global _NC
    if _NC is None:
        _NC = build()
    return _NC


def _pmajor(x, f):
    # [B, C, n] -> [128, B, C, f] with n = p*f + j
    return np.ascontiguousarray(
        x.reshape(B, C, P, f).transpose(2, 0, 1, 3))


def make_in_maps(inputs):
    full = {k: np.asarray(inputs[k], dtype=np.float32).reshape(B, C, N)
            for k in FULL_INPUTS}
    fullp = {k: _pmajor(v, F) for k, v in full.items()}
    in_maps = []
    for r in range(NCORES):
        m = dict(fullp)
        sl = slice(r * NS, (r + 1) * NS)
        m["F_cp_sl"] = _pmajor(full["F_c_previous"][:, :, sl], FS)
        m["F_c_sl"] = _pmajor(full["F_c"][:, :, sl], FS)
        in_maps.append(m)
    return in_maps


def kernel(**inputs):
    nc = _get_nc()
    res = run_bass_kernel_spmd(nc, make_in_maps(inputs),
                               core_ids=list(range(NCORES)))
    # out_sl arrives partition-major [128, B, C, FS]; restore [B, C, NS]
    return np.concatenate(
        [res.results[r]["out_sl"].transpose(1, 2, 0, 3).reshape(B, C, NS)
         for r in range(NCORES)], axis=2)
